# revision 1
# baseline (speedup 1.0000x reference)
"""Trainium2 Bass kernel for nn_Block_69191923139027 (dense_transformer).

Sharding: 8 cores; core k owns Feebler/Booster rows i in [8k, 8k+8), which
is exactly tokens [256k, 256k+256) per batch, so the Booster needs only
locally-computed h (no h AllGather). Two tiny AllGathers stitch the global
k/v sums and softmax denominators.

v4: feebler j-reduce via ones2-stationary matmuls on fp16 products
(x/fw host-cast to fp16), h -> token-major via PE transposes, batched
LN/FFN over all 1024 local tokens, booster broadcast via SEL-matrix
matmuls on fp16 h, fp16 bw and fp16 output (host casts to fp32).

Self-contained: hardcodes all shapes; no sibling imports.
"""

import numpy as np

import concourse.bacc as bacc
import concourse.mybir as mybir
import concourse.tile as tile
from concourse.bass_utils import run_bass_kernel_spmd

N_CORES = 8
B, T, SD, NE = 4, 2048, 64, 4096
H, HS, FH = 8, 8, 256
EPS = 1e-5
IPC = SD // N_CORES          # 8 feebler rows per core
TLOC = B * IPC * 32          # 1024 local tokens; hT col = b*256 + i*32 + a
DT = mybir.dt.float32
F16 = mybir.dt.float16
RG = [list(range(N_CORES))]
ESHIFT = 64.0                # softmax logit shift (max |logit| ~ 71)

_CACHE = {}


def _build_nc():
    nc = bacc.Bacc("TRN2", target_bir_lowering=False, debug=False,
                   num_devices=N_CORES)
    A = mybir.AluOpType
    AF = mybir.ActivationFunctionType

    tn = {}
    tn["x"] = nc.dram_tensor("x", [B * IPC * SD, T], F16, kind="ExternalInput")
    tn["fw"] = nc.dram_tensor("fw", [IPC * SD, T], F16, kind="ExternalInput")
    tn["bw"] = nc.dram_tensor("bw", [IPC * SD, T], F16, kind="ExternalInput")
    tn["wqkv"] = nc.dram_tensor("wqkv", [SD, 3 * SD], F16, kind="ExternalInput")
    tn["pw"] = nc.dram_tensor("pw", [SD, SD], DT, kind="ExternalInput")
    tn["pb"] = nc.dram_tensor("pb", [SD, 1], DT, kind="ExternalInput")
    tn["l1g"] = nc.dram_tensor("l1g", [SD, 1], DT, kind="ExternalInput")
    tn["l1b"] = nc.dram_tensor("l1b", [SD, 1], DT, kind="ExternalInput")
    tn["l2g"] = nc.dram_tensor("l2g", [SD, 1], DT, kind="ExternalInput")
    tn["l2b"] = nc.dram_tensor("l2b", [SD, 1], DT, kind="ExternalInput")
    tn["w1"] = nc.dram_tensor("w1", [SD, FH], F16, kind="ExternalInput")
    tn["b1h"] = nc.dram_tensor("b1h", [128, 2], DT, kind="ExternalInput")
    tn["w2"] = nc.dram_tensor("w2", [FH, SD], F16, kind="ExternalInput")
    tn["b2"] = nc.dram_tensor("b2", [SD, 1], DT, kind="ExternalInput")
    tn["eye64"] = nc.dram_tensor("eye64", [64, 64], DT, kind="ExternalInput")
    tn["selh"] = nc.dram_tensor("selh", [32, 2048], F16, kind="ExternalInput")
    tn["ones32"] = nc.dram_tensor("ones32", [128, 512], F16,
                                  kind="ExternalInput")
    out = nc.dram_tensor("out", [B * IPC * SD, T], F16, kind="ExternalOutput")
    if _CACHE.get("debug"):
        tn["dbg_h"] = nc.dram_tensor("dbg_h", [32, T], DT,
                                     kind="ExternalOutput")
        tn["dbg_hT"] = nc.dram_tensor("dbg_hT", [64, TLOC], DT,
                                      kind="ExternalOutput")
        tn["dbg_h2h"] = nc.dram_tensor("dbg_h2h", [32, T], F16,
                                       kind="ExternalOutput")
        tn["dbg_eT"] = nc.dram_tensor("dbg_eT", [64, TLOC], DT,
                                      kind="ExternalOutput")
        tn["dbg_sm"] = nc.dram_tensor("dbg_sm", [64, 16], DT,
                                      kind="ExternalOutput")

    with tile.TileContext(nc) as tc:
        _body(nc, tc, tn, out, A, AF)
    nc.compile()
    return nc


def _body(nc, tc, tn, out, A, AF):
    X = mybir.AxisListType.X

    with tc.tile_pool(name="wconst", bufs=1) as wp, \
         tc.tile_pool(name="mid", bufs=1) as mp, \
         tc.tile_pool(name="bwpool", bufs=1) as bwp, \
         tc.tile_pool(name="dram", bufs=1, space="DRAM") as dp:

        # ---- on-chip constants ----
        # ones32[:, (b*4+m)*32 : +32]: reduction matrix mapping the
        # (i-parity, j) partition sums of chunk (b, m) onto h_sb rows
        # b*8+2m / b*8+2m+1 of a 32-row accumulating PSUM tile.
        ones32 = wp.tile([128, 512], F16, tag="ones32")
        nc.sync.dma_start(ones32[:], tn["ones32"][:])
        ones64 = wp.tile([SD, 1], DT, tag="ones64")
        nc.vector.memset(ones64[:], 1.0 / SD)
        ones1r = wp.tile([1, SD], DT, tag="ones1r")
        nc.vector.memset(ones1r[:], 1.0)
        epsv = wp.tile([1, 1], DT, tag="epsv")
        nc.vector.memset(epsv[:], EPS)
        neg64 = wp.tile([64, 1], DT, tag="neg64")
        nc.vector.memset(neg64[:], -ESHIFT)
        selh = wp.tile([32, 2048], F16, tag="selh")

        # ---- small weights (DMAs emitted inside the feebler scope so the
        # x/fw stream goes first) ----
        wtiles = {}

        def wtile(name, shape, src, dtype=DT):
            t = wp.tile(shape, dtype, tag=name, name=f"w_{name}")
            nc.sync.dma_start(t[:], src)
            wtiles[name] = t
            return t

        def emit_small_weights():
            wtile("wqkv", [SD, 3 * SD], tn["wqkv"][:], F16)
            wtile("pw", [SD, SD], tn["pw"][:])
            wtile("pb", [SD, 1], tn["pb"][:])
            wtile("l1g", [SD, 1], tn["l1g"][:])
            wtile("l1b", [SD, 1], tn["l1b"][:])
            wtile("l2g", [SD, 1], tn["l2g"][:])
            wtile("l2b", [SD, 1], tn["l2b"][:])
            wtile("w1", [SD, FH], tn["w1"][:], F16)
            wtile("b1h", [128, 2], tn["b1h"][:])
            wtile("w2a", [128, SD], tn["w2"][0:128, :], F16)
            wtile("w2b", [128, SD], tn["w2"][128:256, :], F16)
            wtile("b2", [SD, 1], tn["b2"][:])
            wtile("eye64", [64, 64], tn["eye64"][:])

        # ---- persistent mid-size tiles ----
        h_sb = mp.tile([32, T], DT, tag="h_sb")       # row b*8+i, col a*64+s
        hT = mp.tile([64, TLOC], DT, tag="hT")        # row s, col b*256+i*32+a
        y1 = mp.tile([64, TLOC], F16, tag="y1")
        eT = mp.tile([64, TLOC], DT, tag="eT")
        part = mp.tile([64, B], DT, tag="part")
        zp = mp.tile([64, B], DT, tag="zp")
        sc = mp.tile([64, B], DT, tag="sc")
        h2h = mp.tile([32, T], F16, tag="h2h")        # final h, i-major

        hT4 = hT[:].rearrange("s (b i a) -> s b i a", b=B, i=IPC)

        # layer norm over channel dim s (partitions) for W token columns.
        # Own scoped PSUM pools; broadcasts via K=1 matmuls on the PE.
        def layer_norm(y_out, h_ap, g, bta, W, lnp, tag):
            sq = lnp.tile([64, W], DT, tag="ln_sq")
            nc.vector.tensor_mul(sq[:], h_ap, h_ap)
            mean_sb = lnp.tile([1, W], DT, tag="ln_mean_sb")
            rstd = lnp.tile([1, W], DT, tag="ln_rstd")
            with tc.tile_pool(name=f"ps_{tag}a", bufs=1, space="PSUM") as pa:
                stats_ps = pa.tile([1, 2 * W], DT, tag="ln_stats")
                for c in range(0, W, 512):
                    sl = slice(c, min(c + 512, W))
                    slm = slice(W + c, W + min(c + 512, W))
                    nc.tensor.matmul(stats_ps[:, sl], ones64[:], h_ap[:, sl],
                                     start=True, stop=True)
                    nc.tensor.matmul(stats_ps[:, slm], ones64[:], sq[:, sl],
                                     start=True, stop=True)
                mean_ps = stats_ps[:, 0:W]
                msq_ps = stats_ps[:, W:2 * W]
                nc.scalar.copy(mean_sb[:], mean_ps)
                mbsq = lnp.tile([1, W], DT, tag="ln_mbsq")
                nc.vector.tensor_mul(mbsq[:], mean_sb[:], mean_sb[:])
                var = lnp.tile([1, W], DT, tag="ln_var")
                nc.vector.tensor_sub(var[:], msq_ps, mbsq[:])
                nc.vector.tensor_scalar_add(var[:], var[:], epsv[0:1, 0:1])
                rvar = lnp.tile([1, W], DT, tag="ln_rvar")
                nc.vector.reciprocal_approx_fast(rvar[:], var[:])
                nc.scalar.sqrt(rstd[:], rvar[:])
            with tc.tile_pool(name=f"ps_{tag}b", bufs=1, space="PSUM") as pb:
                meanb = pb.tile([64, W], DT, tag="ln_meanb")
                rstdb = pb.tile([64, W], DT, tag="ln_rstdb")
                for c in range(0, W, 512):
                    sl = slice(c, min(c + 512, W))
                    nc.tensor.matmul(meanb[:, sl], ones1r[:],
                                     mean_sb[:, sl], start=True, stop=True)
                    nc.tensor.matmul(rstdb[:, sl], ones1r[:],
                                     rstd[:, sl], start=True, stop=True)
                ymm = lnp.tile([64, W], DT, tag="ln_ymm")
                nc.vector.tensor_sub(ymm[:], h_ap, meanb[:])
                nc.vector.scalar_tensor_tensor(y_out, ymm[:], g[:, 0:1],
                                               rstdb[:], op0=A.mult,
                                               op1=A.mult)
                nc.vector.tensor_scalar_add(y_out, y_out, bta[:, 0:1])

        # ======== Phase A: stream x in two t'-halves; feebler + transpose
        with nc.named_scope("feebler"), \
             tc.tile_pool(name="fw", bufs=1) as fwp, \
             tc.tile_pool(name="xin", bufs=2) as xp, \
             tc.tile_pool(name="prod", bufs=2) as prp, \
             tc.tile_pool(name="psA", bufs=1, space="PSUM") as psA:
            fwt = fwp.tile([128, 4 * T], F16, tag="fwt")
            nc.sync.dma_start(
                fwt[:], tn["fw"][:].rearrange("(m p) t -> p m t", m=4))
            fwv = fwt[:].rearrange("p (m t) -> p m t", m=4)
            for c in range(2):          # t' halves of 1024
                csl = slice(c * 1024, (c + 1) * 1024)
                xs = xp.tile([128, 16384], F16, tag="xs")
                for b in range(B):
                    nc.sync.dma_start(
                        xs[:, b * 4096:(b + 1) * 4096],
                        tn["x"][b * 512:(b + 1) * 512, csl].rearrange(
                            "(q p) t -> p q t", q=4))
                if c == 0:
                    emit_small_weights()
                h_ps = psA.tile([32, 1024], DT, tag="h_ps", bufs=2)
                for b in range(B):
                    prod = prp.tile([128, 4096], F16, tag="prod")
                    nc.vector.tensor_mul(
                        prod[:].rearrange("p (m t) -> p m t", m=4),
                        xs[:, b * 4096:(b + 1) * 4096].rearrange(
                            "p (m t) -> p m t", m=4),
                        fwv[:, :, csl])
                    for m in range(4):
                        o32 = ones32[:, (b * 4 + m) * 32:
                                     (b * 4 + m + 1) * 32]
                        for u in range(2):
                            nc.tensor.matmul(
                                h_ps[:, u * 512:(u + 1) * 512],
                                o32,
                                prod[:, m * 1024 + u * 512:
                                     m * 1024 + (u + 1) * 512],
                                start=(b == 0 and m == 0),
                                stop=(b == 3 and m == 3))
                nc.scalar.copy(h_sb[:, csl], h_ps[:])
                # transposes for finished 128-col chunks of this half
                for cc in range(8):
                    cidx = c * 8 + cc
                    col = cidx * 128
                    tp = psA.tile([128, 32], DT, tag="tp", bufs=2)
                    nc.tensor.transpose(tp[:], h_sb[:, col:col + 128],
                                        wtiles["eye64"][0:32, 0:32])
                    for da in range(2):
                        nc.scalar.copy(
                            hT4[:, :, :, 2 * cidx + da],
                            tp[da * 64:(da + 1) * 64, :].rearrange(
                                "s (b i) -> s b i", b=B))

        if "dbg_h" in tn:
            nc.sync.dma_start(tn["dbg_h"][:], h_sb[:])
            nc.sync.dma_start(tn["dbg_hT"][:], hT[:])

        # prefetch booster weights into the post-stream DMA gap
        bwt = bwp.tile([128, 4 * T], F16, tag="bwt")
        nc.sync.dma_start(
            bwt[:], tn["bw"][:].rearrange("(m p) t -> p m t", m=4))

        # ======== LN1 -> y1; AG1 (y sums); q; exp; AG2 (denoms)
        cc1i = dp.tile([64, B], DT, tag="cc1i")
        cc1o = dp.tile([512, B], DT, tag="cc1o", addr_space="Shared")
        cc2i = dp.tile([64, B], DT, tag="cc2i")
        cc2o = dp.tile([512, B], DT, tag="cc2o", addr_space="Shared")
        with nc.named_scope("attn"), \
             tc.tile_pool(name="ln1t", bufs=1) as lnp1:
            layer_norm(y1[:], hT[:], wtiles["l1g"], wtiles["l1b"],
                       TLOC, lnp1, "l1")
            nc.vector.tensor_reduce(
                part[:], y1[:].rearrange("s (b t) -> s b t", b=B),
                axis=X, op=A.add)
            nc.sync.dma_start(cc1i[:], part[:])
            nc.gpsimd.collective_compute("AllGather", A.bypass,
                                         ins=[cc1i[:]], outs=[cc1o[:]],
                                         replica_groups=RG)
            with tc.tile_pool(name="psL", bufs=1, space="PSUM") as psL:
                # q overlaps the AllGather
                q_ps = psL.tile([64, TLOC], DT, tag="q_ps")
                for u in range(2):
                    sl = slice(u * 512, (u + 1) * 512)
                    nc.tensor.matmul(q_ps[:, sl], wtiles["wqkv"][:, 0:64],
                                     y1[:, sl], start=True, stop=True)
                gath = lnp1.tile([64, 4 * N_CORES], DT, tag="gath")
                nc.sync.dma_start(
                    gath[:].rearrange("p (s r) -> p s r", s=B),
                    cc1o[:].rearrange("(r p) s -> p s r", r=N_CORES))
                ysum = lnp1.tile([64, B], DT, tag="ysum")
                nc.vector.tensor_reduce(
                    ysum[:], gath[:].rearrange("p (s r) -> p s r", s=B),
                    axis=X, op=A.add)
                ysumh = lnp1.tile([64, B], F16, tag="ysumh")
                nc.scalar.copy(ysumh[:], ysum[:])
                kv_ps = psL.tile([128, B], DT, tag="kv_ps")
                nc.tensor.matmul(kv_ps[:], wtiles["wqkv"][:, 64:192],
                                 ysumh[:], start=True, stop=True)
                kvg = lnp1.tile([128, B], DT, tag="kvg")
                nc.scalar.copy(kvg[:], kv_ps[:])
                for b in range(B):
                    sl = slice(b * 256, (b + 1) * 256)
                    nc.scalar.activation(eT[:, sl], q_ps[:, sl], AF.Exp,
                                         bias=neg64[:, 0:1],
                                         scale=kvg[0:64, b:b + 1],
                                         accum_out=zp[:, b:b + 1])
                nc.sync.dma_start(cc2i[:], zp[:])
                nc.gpsimd.collective_compute("AllGather", A.bypass,
                                             ins=[cc2i[:]], outs=[cc2o[:]],
                                             replica_groups=RG)
                gath2 = lnp1.tile([64, 4 * N_CORES], DT, tag="gath2")
                nc.sync.dma_start(
                    gath2[:].rearrange("p (s r) -> p s r", s=B),
                    cc2o[:].rearrange("(r p) s -> p s r", r=N_CORES))
                zg = lnp1.tile([64, B], DT, tag="zg")
                nc.vector.tensor_reduce(
                    zg[:], gath2[:].rearrange("p (s r) -> p s r", s=B),
                    axis=X, op=A.add)
                rz = lnp1.tile([64, B], DT, tag="rz")
                nc.vector.reciprocal(rz[:], zg[:])
                cvg = lnp1.tile([64, B], DT, tag="cvg")
                nc.scalar.copy(cvg[:], kvg[64:128, :])
                nc.vector.tensor_mul(sc[:], cvg[:], rz[:])
                if "dbg_sm" in tn:
                    nc.sync.dma_start(tn["dbg_eT"][:], eT[:])
                    nc.sync.dma_start(tn["dbg_sm"][:, 0:4], zp[:])
                    nc.sync.dma_start(tn["dbg_sm"][:, 4:8], zg[:])
                    nc.sync.dma_start(tn["dbg_sm"][:, 8:12], sc[:])
                    nc.sync.dma_start(tn["dbg_sm"][:, 12:16], ysum[:])

        # ======== proj -> LN2 -> FFN, batched over all 1024 tokens
        with nc.named_scope("midp"), \
             tc.tile_pool(name="pot", bufs=1) as pot:
            with tc.tile_pool(name="psM1", bufs=1, space="PSUM") as pm1:
                pj = pm1.tile([64, TLOC], DT, tag="pj")
                for b in range(B):
                    sl = slice(b * 256, (b + 1) * 256)
                    pwb = pot.tile([64, 64], DT, tag="pwb", bufs=2)
                    nc.vector.tensor_scalar_mul(pwb[:], wtiles["pw"][:],
                                                sc[:, b:b + 1])
                    nc.tensor.matmul(pj[:, sl], pwb[:], eT[:, sl],
                                     start=True, stop=True)
                nc.vector.scalar_tensor_tensor(hT[:], pj[:],
                                               wtiles["pb"][:, 0:1], hT[:],
                                               op0=A.add, op1=A.add)
                y2 = pot.tile([64, TLOC], F16, tag="y2")
            layer_norm(y2[:], hT[:], wtiles["l2g"], wtiles["l2b"],
                       TLOC, pot, "l2")
            with tc.tile_pool(name="psM2", bufs=1, space="PSUM") as pm2:
                f1a = pm2.tile([128, TLOC], DT, tag="f1a")
                f1b = pm2.tile([128, TLOC], DT, tag="f1b")
                for u in range(2):
                    sl = slice(u * 512, (u + 1) * 512)
                    nc.tensor.matmul(f1a[:, sl], wtiles["w1"][:, 0:128],
                                     y2[:, sl], start=True, stop=True)
                    nc.tensor.matmul(f1b[:, sl], wtiles["w1"][:, 128:256],
                                     y2[:, sl], start=True, stop=True)
                r1a = pot.tile([128, TLOC], F16, tag="r1a")
                r1b = pot.tile([128, TLOC], F16, tag="r1b")
                nc.scalar.activation(r1a[:], f1a[:], AF.Relu,
                                     bias=wtiles["b1h"][:, 0:1])
                nc.scalar.activation(r1b[:], f1b[:], AF.Relu,
                                     bias=wtiles["b1h"][:, 1:2])
                f2 = pm2.tile([64, TLOC], DT, tag="f2")
                for u in range(2):
                    sl = slice(u * 512, (u + 1) * 512)
                    nc.tensor.matmul(f2[:, sl], wtiles["w2a"][:],
                                     r1a[:, sl], start=True, stop=False)
                    nc.tensor.matmul(f2[:, sl], wtiles["w2b"][:],
                                     r1b[:, sl], start=False, stop=True)
                nc.vector.scalar_tensor_tensor(hT[:], f2[:],
                                               wtiles["b2"][:, 0:1], hT[:],
                                               op0=A.add, op1=A.add)

        # ======== Booster: back-transpose h to local-row-major (DRAM fold),
        # broadcast rows via SEL matmuls, multiply with bw, stream out.
        hr_d = dp.tile([32, T], F16, tag="hr_d")
        with nc.named_scope("booster"), \
             tc.tile_pool(name="bst", bufs=1) as bst, \
             tc.tile_pool(name="psB", bufs=1, space="PSUM") as psB:
            nc.sync.dma_start(selh[:], tn["selh"][:])
            eye64 = wtiles["eye64"]
            for b in range(B):
                for cq in range(2):
                    tpb = psB.tile([128, 64], DT, tag="tpb", bufs=2)
                    col = b * 256 + cq * 128
                    nc.tensor.transpose(tpb[:], hT[:, col:col + 128],
                                        eye64[:])
                    stage = bst.tile([128, 64], F16, tag="stage", bufs=2)
                    nc.scalar.copy(stage[:], tpb[:])
                    r0 = b * 8 + cq * 4
                    nc.sync.dma_start(
                        hr_d[r0:r0 + 4, :].rearrange("i (a s) -> (i a) s",
                                                     a=32),
                        stage[:])
            nc.sync.dma_start(h2h[:], hr_d[:])
            if "dbg_h2h" in tn:
                nc.sync.dma_start(tn["dbg_h2h"][:], h2h[:])
            for b in range(B):
                for m in range(4):
                    pr = bst.tile([128, T], F16, tag="pr", bufs=3)
                    for half in range(2):
                        bc = psB.tile([128, 1024], DT, tag="bc", bufs=3)
                        hsl = slice(half * 1024, (half + 1) * 1024)
                        for u in range(2):
                            us = slice(half * 1024 + u * 512,
                                       half * 1024 + (u + 1) * 512)
                            nc.tensor.matmul(
                                bc[:, u * 512:(u + 1) * 512],
                                selh[:, (b * 4 + m) * 128:
                                     (b * 4 + m) * 128 + 128],
                                h2h[:, us], start=True, stop=True)
                        bch = bst.tile([128, 1024], F16,
                                       tag="bch", bufs=3)
                        nc.scalar.copy(bch[:], bc[:])
                        nc.vector.tensor_mul(
                            pr[:, hsl], bwt[:, m * T + half * 1024:
                                            m * T + (half + 1) * 1024],
                            bch[:])
                    r0 = (b * 8 + 2 * m) * 64
                    nc.sync.dma_start(out[r0:r0 + 128, :], pr[:])


def _prep_host(inputs):
    """Host-side prep: shard x/fw/bw per core (fp16); pack small weights."""
    f32 = np.float32
    g = {k: np.asarray(v, dtype=f32) for k, v in inputs.items()}
    x = g["x"].reshape(B, SD, SD, T)          # flat view (b, i, j, t')
    fw, bw = g["feebler_w"], g["booster_w"]
    wq, wk, wv = g["wq"], g["wk"], g["wv"]
    wqkv = np.concatenate([w.transpose(1, 0, 2).reshape(SD, SD)
                           for w in (wq, wk, wv)], axis=1)  # [64, 192]
    shared = {
        "wqkv": np.ascontiguousarray(wqkv).astype(np.float16),
        "pw": np.ascontiguousarray(g["proj_w"]),
        "pb": g["proj_b"].reshape(SD, 1).copy(),
        "l1g": g["ln1_g"].reshape(SD, 1).copy(),
        "l1b": g["ln1_b"].reshape(SD, 1).copy(),
        "l2g": g["ln2_g"].reshape(SD, 1).copy(),
        "l2b": g["ln2_b"].reshape(SD, 1).copy(),
        "w1": np.ascontiguousarray(g["w1"]).astype(np.float16),
        "b1h": np.ascontiguousarray(g["b1"].reshape(2, 128).T),
        "w2": np.ascontiguousarray(g["w2"]).astype(np.float16),
        "b2": g["b2"].reshape(SD, 1).copy(),
        "eye64": np.eye(64, dtype=f32),
    }
    sel = np.zeros((32, 2048), np.float16)
    for b in range(B):
        for m in range(4):
            c0 = (b * 4 + m) * 128
            sel[b * 8 + 2 * m, c0:c0 + 64] = 1.0
            sel[b * 8 + 2 * m + 1, c0 + 64:c0 + 128] = 1.0
    shared["selh"] = sel
    o32 = np.zeros((128, 512), np.float16)
    for b in range(B):
        for m in range(4):
            c0 = (b * 4 + m) * 32
            o32[0:64, c0 + b * 8 + 2 * m] = 1.0
            o32[64:128, c0 + b * 8 + 2 * m + 1] = 1.0
    shared["ones32"] = o32
    in_maps = []
    for k in range(N_CORES):
        i0 = k * IPC
        m = dict(shared)
        m["x"] = np.ascontiguousarray(
            x[:, i0:i0 + IPC].reshape(B * IPC * SD, T)).astype(np.float16)
        m["fw"] = np.ascontiguousarray(
            fw[i0:i0 + IPC].reshape(IPC * SD, T)).astype(np.float16)
        # booster output is sharded over j (rev[b,i,j]=bw[i,j]*hr[b,j]):
        # rows (j_loc, i) so the broadcast h row per 64-row group is local
        m["bw"] = np.ascontiguousarray(
            bw[:, i0:i0 + IPC].transpose(1, 0, 2).reshape(
                IPC * SD, T)).astype(np.float16)
        in_maps.append(m)
    return in_maps


def _get_nc():
    if "nc" not in _CACHE:
        _CACHE["nc"] = _build_nc()
    return _CACHE["nc"]


def run(inputs, trace=False, **kw):
    nc = _get_nc()
    in_maps = _prep_host(inputs)
    res = run_bass_kernel_spmd(nc, in_maps, core_ids=list(range(N_CORES)),
                               trace=trace, **kw)
    full = np.empty((B, SD, SD, T), dtype=np.float32)
    for k in range(N_CORES):
        i0 = k * IPC
        co = res.results[k]["out"].astype(np.float32).reshape(B, IPC, SD, T)
        full[:, :, i0:i0 + IPC] = co.transpose(0, 2, 1, 3)
    return full.reshape(B, T, NE), res


def kernel(**inputs):
    out, _ = run(inputs)
    return out



# revision 21
# speedup vs baseline: 1.1122x; 1.1122x over previous
"""Trainium2 Bass kernel for nn_Block_69191923139027 (dense_transformer).

Sharding: 8 cores; core k owns Feebler/Booster rows i in [8k, 8k+8), which
is exactly tokens [256k, 256k+256) per batch, so the Booster needs only
locally-computed h (no h AllGather). Two tiny AllReduces stitch the global
k/v sums and softmax denominators.

v5: quarter-pipelined x stream keeps PE warm (2.4GHz), constant [128,2]
feebler stationary, AllReduce of pre-projected k/v partials, preloaded
activation tables, 16-bit stationaries/moving operands everywhere,
softmax 1/z applied to e (bounded <=1) instead of the proj weights,
booster multiplies read PSUM directly split across DVE and GpSimd.

Self-contained: hardcodes all shapes; no sibling imports.
"""

import numpy as np

import concourse.bacc as bacc
import concourse.mybir as mybir
import concourse.tile as tile
from concourse.bass_utils import run_bass_kernel_spmd

N_CORES = 8
B, T, SD, NE = 4, 2048, 64, 4096
H, HS, FH = 8, 8, 256
EPS = 1e-5
IPC = SD // N_CORES          # 8 feebler rows per core
TLOC = B * IPC * 32          # 1024 local tokens; hT col = b*256 + a*8 + i
DT = mybir.dt.float32
F16 = mybir.dt.float16
RG = [list(range(N_CORES))]
ESHIFT = 64.0                # softmax logit shift (max |logit| ~ 71)

_CACHE = {}


def _build_nc():
    nc = bacc.Bacc("TRN2", target_bir_lowering=False, debug=False,
                   num_devices=N_CORES)
    A = mybir.AluOpType
    AF = mybir.ActivationFunctionType

    tn = {}
    tn["x"] = nc.dram_tensor("x", [B * IPC * SD, T], F16, kind="ExternalInput")
    tn["fw"] = nc.dram_tensor("fw", [IPC * SD, T], F16, kind="ExternalInput")
    tn["bw"] = nc.dram_tensor("bw", [IPC * SD, T], F16, kind="ExternalInput")
    # all small weights packed into two tensors (2 DMA issues, not 13)
    tn["pk32"] = nc.dram_tensor("pk32", [128, 134], DT, kind="ExternalInput")
    tn["pk16"] = nc.dram_tensor("pk16", [128, 3136], F16,
                                kind="ExternalInput")
    out = nc.dram_tensor("out", [B * IPC * SD, T], F16, kind="ExternalOutput")
    if _CACHE.get("debug"):
        tn["dbg_h"] = nc.dram_tensor("dbg_h", [32, T], DT,
                                     kind="ExternalOutput")
        tn["dbg_hT"] = nc.dram_tensor("dbg_hT", [64, TLOC], DT,
                                      kind="ExternalOutput")
        tn["dbg_y1"] = nc.dram_tensor("dbg_y1", [64, TLOC], F16,
                                      kind="ExternalOutput")
        tn["dbg_e2"] = nc.dram_tensor("dbg_e2", [64, TLOC], F16,
                                      kind="ExternalOutput")
        tn["dbg_kv"] = nc.dram_tensor("dbg_kv", [128, 3 * B], DT,
                                      kind="ExternalOutput")
        tn["dbg_h3"] = nc.dram_tensor("dbg_h3", [64, TLOC], DT,
                                      kind="ExternalOutput")
        tn["dbg_h2h"] = nc.dram_tensor("dbg_h2h", [32, T], F16,
                                       kind="ExternalOutput")

    with tile.TileContext(nc) as tc:
        _body(nc, tc, tn, out, A, AF)
    nc.compile()
    return nc


def _body(nc, tc, tn, out, A, AF):
    X = mybir.AxisListType.X

    with tc.tile_pool(name="wconst", bufs=1) as wp, \
         tc.tile_pool(name="mid", bufs=1) as mp, \
         tc.tile_pool(name="bwpool", bufs=1) as bwp, \
         tc.tile_pool(name="dram", bufs=1, space="DRAM") as dp:

        # ---- on-chip constants ----
        ones64 = wp.tile([SD, 2], F16, tag="ones64")
        nc.vector.memset(ones64[:], 1.0 / SD)
        ones1r = wp.tile([1, SD], F16, tag="ones1r")
        nc.vector.memset(ones1r[:], 1.0)
        epsv = wp.tile([1, 1], DT, tag="epsv")
        nc.vector.memset(epsv[:], EPS)

        # preload activation tables (Exp/Sqrt/Square/Relu/Identity) so no
        # ACT_TABLE_LOAD lands on the critical path later.
        warm = wp.tile([1, 2], DT, tag="warm")
        nc.vector.memset(warm[:], 1.0)
        for fn in (AF.Identity, AF.Square, AF.Sqrt, AF.Relu, AF.Exp):
            nc.scalar.activation(warm[:], warm[:], fn)

        # ---- small weights, two packed tiles (2 DMA issues, not 13) ----
        pk32 = wp.tile([128, 134], DT, tag="pk32")
        pk16 = wp.tile([128, 3136], F16, tag="pk16")

        def emit_small_weights():
            nc.sync.dma_start(pk32[:], tn["pk32"][:])
            nc.sync.dma_start(pk16[:], tn["pk16"][:])

        def wt(name):
            m32 = {"eye32": (0, 32, 0, 32), "eye64": (0, 64, 0, 64),
                   "pw": (0, 64, 64, 128), "kvb": (0, 128, 128, 129),
                   "b1a": (0, 128, 129, 130), "b1b": (0, 128, 130, 131),
                   "bq": (0, 64, 131, 132), "pb": (0, 64, 132, 133),
                   "b2": (0, 64, 133, 134)}
            m16 = {"w2a": (0, 128, 2048, 2112), "w2b": (0, 128, 2112, 2176),
                   "wq": (0, 64, 2176, 2240), "wkv": (0, 64, 2240, 2368),
                   "w1a": (0, 64, 2368, 2496), "w1b": (0, 64, 2496, 2624)}
            if name in m32:
                r0, r1, c0, c1 = m32[name]
                return pk32[r0:r1, c0:c1]
            r0, r1, c0, c1 = m16[name]
            return pk16[r0:r1, c0:c1]

        def selh_bm(idx):
            return pk16[0:32, idx * 128:(idx + 1) * 128]

        # ---- persistent mid-size tiles ----
        h_sb = mp.tile([32, T], DT, tag="h_sb")       # row b*8+i, col a*64+s
        hT = mp.tile([64, TLOC], DT, tag="hT")        # row s, col b*256+a*8+i
        y1 = mp.tile([64, TLOC], F16, tag="y1")
        eT = mp.tile([64, TLOC], DT, tag="eT")
        e2 = mp.tile([64, TLOC], F16, tag="e2")
        zp = mp.tile([64, B], DT, tag="zp")
        h2h = mp.tile([32, T], F16, tag="h2h")        # final h, i-major

        # hT col = b*256 + i*32 + a  (i-major inside each batch block)
        hT4 = hT[:].rearrange("s (b i a) -> s b i a", b=B, i=IPC)

        # channel-dim layer norm (affine folded into downstream weights):
        # y_out = (h - mean) * rsqrt(var + eps), over W token columns.
        def layer_norm(y_out, h_ap, W, lnp, tag):
            h16 = lnp.tile([64, W], F16, tag=f"ln_h16_{tag}")
            nc.scalar.copy(h16[:], h_ap)
            sq = lnp.tile([64, W], F16, tag=f"ln_sq_{tag}")
            nc.vector.tensor_mul(sq[:], h_ap, h_ap)
            with tc.tile_pool(name=f"ps_{tag}", bufs=1, space="PSUM") as pp:
                st = pp.tile([1, 2 * W], DT, tag=f"ln_st_{tag}")
                for c in range(0, W, 512):
                    sl = slice(c, min(c + 512, W))
                    slm = slice(W + c, W + min(c + 512, W))
                    nc.tensor.matmul(st[:, sl], ones64[:, 0:1], h16[:, sl],
                                     start=True, stop=True)
                    nc.tensor.matmul(st[:, slm], ones64[:, 1:2], sq[:, sl],
                                     start=True, stop=True)
                mean_h = lnp.tile([1, W], F16, tag=f"ln_mh_{tag}")
                nc.scalar.copy(mean_h[:], st[:, 0:W])
                mbsq = lnp.tile([1, W], DT, tag=f"ln_mbsq_{tag}")
                nc.scalar.activation(mbsq[:], st[:, 0:W], AF.Square)
                var = lnp.tile([1, W], DT, tag=f"ln_var_{tag}")
                nc.vector.scalar_tensor_tensor(var[:], st[:, W:2 * W],
                                               epsv[0:1, 0:1], mbsq[:],
                                               op0=A.add, op1=A.subtract)
                rvar = lnp.tile([1, W], DT, tag=f"ln_rvar_{tag}")
                nc.vector.reciprocal_approx_fast(rvar[:], var[:])
                rstd_h = lnp.tile([1, W], F16, tag=f"ln_rsh_{tag}")
                nc.scalar.sqrt(rstd_h[:], rvar[:])
                mrb = pp.tile([128, W], DT, tag=f"ln_mrb_{tag}")
                for c in range(0, W, 512):
                    sl = slice(c, min(c + 512, W))
                    nc.tensor.matmul(mrb[0:64, sl], ones1r[:],
                                     mean_h[:, sl], start=True, stop=True)
                    nc.tensor.matmul(mrb[64:128, sl], ones1r[:],
                                     rstd_h[:, sl], start=True, stop=True)
                d = lnp.tile([64, W], DT, tag=f"ln_d_{tag}")
                nc.vector.tensor_sub(d[:], h_ap, mrb[0:64, :])
                nc.vector.tensor_mul(y_out, d[:], mrb[64:128, :])

        # ======== Phase A: stream x in 4 t'-quarters; feebler + transpose
        QW = 512
        with nc.named_scope("feebler"), \
             tc.tile_pool(name="fw", bufs=1) as fwp, \
             tc.tile_pool(name="xin", bufs=2) as xp, \
             tc.tile_pool(name="prod", bufs=2) as prp, \
             tc.tile_pool(name="psA", bufs=1, space="PSUM") as psA:
            fwt = fwp.tile([128, 4 * T], F16, tag="fwt")
            fwv = fwt[:].rearrange("p (m t) -> p m t", m=4)
            for c in range(4):          # t' quarters of 512
                csl = slice(c * QW, (c + 1) * QW)
                # fw quarter then x quarter: q0 compute unblocks after ~3MB
                nc.sync.dma_start(
                    fwv[:, :, csl],
                    tn["fw"][:, csl].rearrange("(m p) t -> p m t", m=4))
                xs = xp.tile([128, B * 4 * QW], F16, tag="xs")
                nc.sync.dma_start(
                    xs[:].rearrange("p (b q t) -> p b q t", b=B, q=4),
                    tn["x"][:, csl].rearrange(
                        "(b q p) t -> p b q t", b=B, q=4))
                if c == 0:
                    emit_small_weights()
                h_ps = psA.tile([32, QW], DT, tag="h_ps", bufs=2)
                for b in range(B):
                    prod = prp.tile([128, 4 * QW], F16, tag="prod")
                    nc.vector.tensor_mul(
                        prod[:].rearrange("p (m t) -> p m t", m=4),
                        xs[:, b * 4 * QW:(b + 1) * 4 * QW].rearrange(
                            "p (m t) -> p m t", m=4),
                        fwv[:, :, csl])
                    for m in range(4):
                        o32 = pk16[:, 2624 + (b * 4 + m) * 32:
                                   2624 + (b * 4 + m + 1) * 32]
                        nc.tensor.matmul(
                            h_ps[:], o32, prod[:, m * QW:(m + 1) * QW],
                            start=(b == 0 and m == 0),
                            stop=(b == 3 and m == 3))
                nc.scalar.copy(h_sb[:, csl], h_ps[:])
                # transposes for finished 128-col chunks of this quarter
                for cc in range(4):
                    cidx = c * 4 + cc
                    col = cidx * 128
                    tp = psA.tile([128, 32], DT, tag="tp", bufs=2)
                    nc.tensor.transpose(tp[:], h_sb[:, col:col + 128],
                                        wt("eye32"))
                    for da in range(2):
                        nc.scalar.copy(
                            hT4[:, :, :, 2 * cidx + da],
                            tp[da * 64:(da + 1) * 64, :].rearrange(
                                "s (b i) -> s b i", b=B))

        if "dbg_h" in tn:
            nc.sync.dma_start(tn["dbg_h"][:], h_sb[:])
            nc.sync.dma_start(tn["dbg_hT"][:], hT[:])

        # prefetch booster weights into the post-stream DMA gap
        bwt = bwp.tile([128, 4 * T], F16, tag="bwt")
        nc.sync.dma_start(
            bwt[:], tn["bw"][:].rearrange("(m p) t -> p m t", m=4))

        # ======== LN1 -> y1; kv partials; AR1; exp; AR2
        cc1i = dp.tile([128, B], DT, tag="cc1i")
        cc1o = dp.tile([128, B], DT, tag="cc1o", addr_space="Shared")
        cc2i = dp.tile([64, B], DT, tag="cc2i")
        cc2o = dp.tile([64, B], DT, tag="cc2o", addr_space="Shared")
        with nc.named_scope("attn"), \
             tc.tile_pool(name="ln1t", bufs=1) as lnp1:
            layer_norm(y1[:], hT[:], TLOC, lnp1, "l1")
            part = lnp1.tile([64, B], DT, tag="part")
            nc.vector.tensor_reduce(
                part[:], y1[:].rearrange("s (b t) -> s b t", b=B),
                axis=X, op=A.add)
            parth = lnp1.tile([64, B], F16, tag="parth")
            nc.scalar.copy(parth[:], part[:])
            with tc.tile_pool(name="psL", bufs=1, space="PSUM") as psL:
                kv_ps = psL.tile([128, B], DT, tag="kv_ps")
                nc.tensor.matmul(kv_ps[:], wt("wkv"),
                                 parth[:], start=True, stop=True)
                kvl = lnp1.tile([128, B], DT, tag="kvl")
                nc.scalar.copy(kvl[:], kv_ps[:])
                nc.sync.dma_start(cc1i[:], kvl[:])
                nc.gpsimd.collective_compute("AllReduce", A.add,
                                             ins=[cc1i[:]], outs=[cc1o[:]],
                                             replica_groups=RG)
                # q overlaps the AllReduce
                q_ps = psL.tile([64, TLOC], DT, tag="q_ps")
                for u in range(2):
                    sl = slice(u * 512, (u + 1) * 512)
                    nc.tensor.matmul(q_ps[:, sl], wt("wq"),
                                     y1[:, sl], start=True, stop=True)
                kvg = lnp1.tile([128, B], DT, tag="kvg")
                nc.sync.dma_start(kvg[:], cc1o[:])
                kvgb = lnp1.tile([128, B], DT, tag="kvgb")
                nc.vector.tensor_scalar_add(kvgb[:], kvg[:],
                                            wt("kvb"))
                ebias = lnp1.tile([64, B], DT, tag="ebias")
                nc.vector.tensor_scalar(ebias[:], kvgb[0:64, :],
                                        wt("bq"), -ESHIFT,
                                        op0=A.mult, op1=A.add)
                for b in range(B):
                    sl = slice(b * 256, (b + 1) * 256)
                    nc.scalar.activation(eT[:, sl], q_ps[:, sl], AF.Exp,
                                         bias=ebias[:, b:b + 1],
                                         scale=kvgb[0:64, b:b + 1],
                                         accum_out=zp[:, b:b + 1])
                nc.sync.dma_start(cc2i[:], zp[:])
                nc.gpsimd.collective_compute("AllReduce", A.add,
                                             ins=[cc2i[:]], outs=[cc2o[:]],
                                             replica_groups=RG)
                zg = lnp1.tile([64, B], DT, tag="zg")
                nc.sync.dma_start(zg[:], cc2o[:])
                cvb = lnp1.tile([64, B], DT, tag="cvb")
                nc.scalar.copy(cvb[:], kvgb[64:128, :])
                rz = lnp1.tile([64, B], DT, tag="rz")
                nc.vector.reciprocal_approx_fast(rz[:], zg[:])
                for b in range(B):
                    sl = slice(b * 256, (b + 1) * 256)
                    nc.vector.tensor_scalar_mul(e2[:, sl], eT[:, sl],
                                                rz[:, b:b + 1])
                pwcv = lnp1.tile([64, 4 * SD], F16, tag="pwcv")
                for b in range(B):
                    nc.vector.tensor_scalar_mul(
                        pwcv[:, b * SD:(b + 1) * SD], wt("pw"),
                        cvb[:, b:b + 1])
                if "dbg_y1" in tn:
                    nc.sync.dma_start(tn["dbg_y1"][:], y1[:])
                    nc.sync.dma_start(tn["dbg_e2"][:], e2[:])
                    nc.sync.dma_start(tn["dbg_kv"][:, 0:B], kvgb[:])
                    nc.sync.dma_start(tn["dbg_kv"][0:64, B:2 * B], zp[:])
                    nc.sync.dma_start(tn["dbg_kv"][0:64, 2 * B:3 * B], zg[:])

        # ======== proj -> LN2 -> FFN, batched over all 1024 tokens
        with nc.named_scope("midp"), \
             tc.tile_pool(name="pot", bufs=1) as pot:
            with tc.tile_pool(name="psM1", bufs=1, space="PSUM") as pm1:
                pj = pm1.tile([64, TLOC], DT, tag="pj")
                for b in range(B):
                    sl = slice(b * 256, (b + 1) * 256)
                    nc.tensor.matmul(pj[:, sl],
                                     pwcv[:, b * SD:(b + 1) * SD],
                                     e2[:, sl], start=True, stop=True)
                nc.vector.scalar_tensor_tensor(hT[:], pj[:],
                                               wt("pb"), hT[:],
                                               op0=A.add, op1=A.add)
            y2 = pot.tile([64, TLOC], F16, tag="y2")
            layer_norm(y2[:], hT[:], TLOC, pot, "l2")
            with tc.tile_pool(name="psM2", bufs=1, space="PSUM") as pm2:
                f1a = pm2.tile([128, TLOC], DT, tag="f1a")
                f1b = pm2.tile([128, TLOC], DT, tag="f1b")
                for u in range(2):
                    sl = slice(u * 512, (u + 1) * 512)
                    nc.tensor.matmul(f1a[:, sl], wt("w1a"),
                                     y2[:, sl], start=True, stop=True)
                    nc.tensor.matmul(f1b[:, sl], wt("w1b"),
                                     y2[:, sl], start=True, stop=True)
                r1a = pot.tile([128, TLOC], F16, tag="r1a")
                r1b = pot.tile([128, TLOC], F16, tag="r1b")
                nc.scalar.activation(r1a[:], f1a[:], AF.Relu,
                                     bias=wt("b1a"))
                nc.scalar.activation(r1b[:], f1b[:], AF.Relu,
                                     bias=wt("b1b"))
                f2 = pm2.tile([64, TLOC], DT, tag="f2")
                for u in range(2):
                    sl = slice(u * 512, (u + 1) * 512)
                    nc.tensor.matmul(f2[:, sl], wt("w2a"),
                                     r1a[:, sl], start=True, stop=False)
                    nc.tensor.matmul(f2[:, sl], wt("w2b"),
                                     r1b[:, sl], start=False, stop=True)
                nc.vector.scalar_tensor_tensor(hT[:], f2[:],
                                               wt("b2"), hT[:],
                                               op0=A.add, op1=A.add)

        # ======== Booster: back-transpose h (DRAM fold) per batch, then
        # per (b,m): SEL-broadcast matmul, DVE/GpSimd multiply straight
        # from PSUM, stream out.
        if "dbg_h3" in tn:
            nc.sync.dma_start(tn["dbg_h3"][:], hT[:])
        hr_d = dp.tile([32, T], F16, tag="hr_d")
        with nc.named_scope("booster"), \
             tc.tile_pool(name="bst", bufs=1) as bst, \
             tc.tile_pool(name="psB", bufs=1, space="PSUM") as psB:
            eye64 = wt("eye64")
            for b in range(B):
                for cq in range(2):
                    tpb = psB.tile([128, 64], DT, tag="tpb", bufs=2)
                    col = b * 256 + cq * 128
                    nc.tensor.transpose(tpb[:], hT[:, col:col + 128],
                                        eye64)
                    stage = bst.tile([128, 64], F16, tag="stage", bufs=2)
                    nc.scalar.copy(stage[:], tpb[:])
                    # chunk cols = (i_rel 4, a 32); each i gives one full
                    # 2048-wide hr_d row.
                    r0 = b * 8 + cq * 4
                    nc.sync.dma_start(
                        hr_d[r0:r0 + 4, :].rearrange("i (a s) -> (i a) s",
                                                     a=32),
                        stage[:])
                nc.sync.dma_start(h2h[b * 8:b * 8 + 8, :],
                                  hr_d[b * 8:b * 8 + 8, :])
            if "dbg_h2h" in tn:
                nc.sync.dma_start(tn["dbg_h2h"][:], h2h[:])
            for b in range(B):
                for m in range(4):
                    pr = bst.tile([128, T], F16, tag="pr", bufs=3)
                    on_dve = (b * 4 + m) % 2 == 0
                    for half in range(2):
                        bc = psB.tile([128, 1024], DT, tag="bc", bufs=3)
                        hsl = slice(half * 1024, (half + 1) * 1024)
                        for u in range(2):
                            us = slice(half * 1024 + u * 512,
                                       half * 1024 + (u + 1) * 512)
                            nc.tensor.matmul(
                                bc[:, u * 512:(u + 1) * 512],
                                selh_bm(b * 4 + m),
                                h2h[:, us], start=True, stop=True)
                        bsl = bwt[:, m * T + half * 1024:
                                  m * T + (half + 1) * 1024]
                        if on_dve:
                            # DVE reads the broadcast straight from PSUM
                            nc.vector.tensor_mul(pr[:, hsl], bsl, bc[:])
                        else:
                            # GpSimd cannot read PSUM: bounce via scalar
                            bch = bst.tile([128, 1024], F16, tag="bch",
                                           bufs=3)
                            nc.scalar.copy(bch[:], bc[:])
                            nc.gpsimd.tensor_mul(pr[:, hsl], bsl, bch[:])
                    r0 = (b * 8 + 2 * m) * 64
                    nc.sync.dma_start(out[r0:r0 + 128, :], pr[:])


def _prep_host(inputs):
    """Host-side prep: shard x/fw/bw per core (fp16); fold LN affines into
    the downstream weights; pack small weights."""
    f32 = np.float32
    g = {k: np.asarray(v, dtype=f32) for k, v in inputs.items()}
    x = g["x"].reshape(B, SD, SD, T)          # flat view (b, i, j, t')
    fw, bw = g["feebler_w"], g["booster_w"]
    wq, wk, wv = g["wq"], g["wk"], g["wv"]
    wqkv = np.concatenate([w.transpose(1, 0, 2).reshape(SD, SD)
                           for w in (wq, wk, wv)], axis=1)  # [64, 192]
    # fold ln1 gamma into wqkv rows; ln1 beta becomes additive biases
    g1 = g["ln1_g"].reshape(SD, 1)
    wqkv_g = wqkv * g1
    bqv = g["ln1_b"] @ wqkv[:, 0:64] * 1.0          # [64] q bias
    bk = g["ln1_b"] @ wqkv[:, 64:128]
    bv = g["ln1_b"] @ wqkv[:, 128:192]
    kvb = np.concatenate([bk, bv]) * float(T)       # [128] k/v sum bias
    # fold ln2 gamma into w1 rows; ln2 beta into b1
    g2 = g["ln2_g"].reshape(SD, 1)
    w1_g = g["w1"] * g2
    b1f = g["b1"] + g["ln2_b"] @ g["w1"]
    b1h = b1f.reshape(2, 128).T.astype(f32)     # [128, 2]
    pk32 = np.zeros((128, 134), f32)
    pk32[0:64, 0:64] = np.eye(64, dtype=f32)
    pk32[0:64, 64:128] = g["proj_w"]
    pk32[:, 128] = kvb
    pk32[:, 129:131] = b1h
    pk32[0:64, 131] = bqv
    pk32[0:64, 132] = g["proj_b"]
    pk32[0:64, 133] = g["b2"]
    sel = np.zeros((32, 2048), np.float16)
    for b in range(B):
        for m in range(4):
            c0 = (b * 4 + m) * 128
            sel[b * 8 + 2 * m, c0:c0 + 64] = 1.0
            sel[b * 8 + 2 * m + 1, c0 + 64:c0 + 128] = 1.0
    pk16 = np.zeros((128, 3136), np.float16)
    pk16[0:32, 0:2048] = sel
    pk16[:, 2048:2112] = g["w2"][0:128, :].astype(np.float16)
    pk16[:, 2112:2176] = g["w2"][128:256, :].astype(np.float16)
    pk16[0:64, 2176:2368] = wqkv_g.astype(np.float16)
    pk16[0:64, 2368:2624] = w1_g.astype(np.float16)
    for b in range(B):
        for m in range(4):
            c0 = 2624 + (b * 4 + m) * 32
            pk16[0:64, c0 + b * 8 + 2 * m] = 1.0
            pk16[64:128, c0 + b * 8 + 2 * m + 1] = 1.0
    shared = {"pk32": pk32, "pk16": np.ascontiguousarray(pk16)}
    in_maps = []
    for k in range(N_CORES):
        i0 = k * IPC
        m = dict(shared)
        m["x"] = np.ascontiguousarray(
            x[:, i0:i0 + IPC].reshape(B * IPC * SD, T)).astype(np.float16)
        m["fw"] = np.ascontiguousarray(
            fw[i0:i0 + IPC].reshape(IPC * SD, T)).astype(np.float16)
        # booster output is sharded over j (rev[b,i,j]=bw[i,j]*hr[b,j]):
        # rows (j_loc, i) so the broadcast h row per 64-row group is local
        m["bw"] = np.ascontiguousarray(
            bw[:, i0:i0 + IPC].transpose(1, 0, 2).reshape(
                IPC * SD, T)).astype(np.float16)
        in_maps.append(m)
    return in_maps


def _get_nc():
    if "nc" not in _CACHE:
        _CACHE["nc"] = _build_nc()
    return _CACHE["nc"]


def run(inputs, trace=False, **kw):
    nc = _get_nc()
    in_maps = _prep_host(inputs)
    res = run_bass_kernel_spmd(nc, in_maps, core_ids=list(range(N_CORES)),
                               trace=trace, **kw)
    full = np.empty((B, SD, SD, T), dtype=np.float32)
    for k in range(N_CORES):
        i0 = k * IPC
        co = res.results[k]["out"].astype(np.float32).reshape(B, IPC, SD, T)
        full[:, :, i0:i0 + IPC] = co.transpose(0, 2, 1, 3)
    return full.reshape(B, T, NE), res


def kernel(**inputs):
    out, _ = run(inputs)
    return out
